# revision 9
# baseline (speedup 1.0000x reference)
"""Capsule routing kernel v2 (Conv1D k=1 -> dynamic routing) for TRN2, 8 cores.

Data-parallel over batch (8 batches/core), 2 groups of 4 batches stacked on
the 128-partition dim as (b,n).  u_hat is never materialized; the routing is
factorized through x:
    s[n,d] = sum_c Q[n,c] W[c,nD+d],  Q = c @ x^T        (PE, big matmuls)
    b[n,t] += sum_c P[c,n] x[c,t],    P = W . vmask      (PE)
Softmax over n runs in ((b,n), t) layout with PE-assisted partition sums
(block-ones matmuls).  All transposes (c -> cT, Q -> Qt) go through the DMA
XBAR transpose (bf16), keeping the PE instruction count at ~196/core vs 736
in the f32r u_hat design.  Everything below psum accumulation runs in bf16
(validated 3.5e-3 rel_fro in numpy).  Host pre-casts/transposes x into both
(c,t) and (t, 4b*c) bf16 layouts, so no on-device layout work is needed.
"""

import contextlib

import numpy as np
import ml_dtypes

import concourse.bass as bass
import concourse.tile as tile
from concourse import bacc, mybir
from concourse.bass_utils import run_bass_kernel_spmd

F32 = mybir.dt.float32
BF16 = mybir.dt.bfloat16
AF = mybir.ActivationFunctionType
AX = mybir.AxisListType

B, C, T = 64, 256, 1024
N, D = 32, 16
O = N * D            # 512
NCORES = 8
BPC = B // NCORES    # 8 batches per core
NG = 2               # groups per core
GB = 4               # batches per group (stacked as (b,n) on 128 partitions)
KC = C // 128        # 2 contraction chunks
MT = T // 128        # 8 t-chunks
OG = O // 128        # 4 o-chunks
EPS = 1e-7


def _build_bass():
    nc = bacc.Bacc(
        "TRN2",
        target_bir_lowering=False,
        debug=False,
        enable_asserts=False,
        num_devices=NCORES,
    )
    xb_d = nc.dram_tensor("xb", [BPC, KC, 128, T], BF16, kind="ExternalInput").ap()
    xt_d = nc.dram_tensor("xt4", [NG, MT, 128, GB * C], BF16, kind="ExternalInput").ap()
    w_d = nc.dram_tensor("wsb", [KC, 128, O], BF16, kind="ExternalInput").ap()
    wt_d = nc.dram_tensor("wt", [OG, 128, C], BF16, kind="ExternalInput").ap()
    e16_d = nc.dram_tensor("e16", [D, 128], BF16, kind="ExternalInput").ap()
    bm_d = nc.dram_tensor("bm", [OG, 128, 128], BF16, kind="ExternalInput").ap()
    dm4_d = nc.dram_tensor("dm4", [128, O], F32, kind="ExternalInput").ap()
    on4_d = nc.dram_tensor("on4", [128, GB], BF16, kind="ExternalInput").ap()
    on4t_d = nc.dram_tensor("on4t", [GB, 128], BF16, kind="ExternalInput").ap()
    on32_d = nc.dram_tensor("on32", [128, N], BF16, kind="ExternalInput").ap()
    id128_d = nc.dram_tensor("id128", [128, 128], F32, kind="ExternalInput").ap()
    cu128_d = nc.dram_tensor("cu128", [128, 128], BF16, kind="ExternalInput").ap()
    out_d = nc.dram_tensor("out", [BPC, N, D], F32, kind="ExternalOutput").ap()

    with tile.TileContext(nc) as tc:
        _kernel_body(tc, out_d, xb_d, xt_d, w_d, wt_d, e16_d, bm_d, dm4_d,
                     on4_d, on4t_d, on32_d, id128_d, cu128_d)
    nc.compile()
    return nc


def _kernel_body(tc, out_d, xb_d, xt_d, w_d, wt_d, e16_d, bm_d, dm4_d,
                 on4_d, on4t_d, on32_d, id128_d, cu128_d):
    nc = tc.nc
    ctx = contextlib.ExitStack()
    with ctx:
        const = ctx.enter_context(tc.tile_pool(name="const", bufs=1))
        xbp = ctx.enter_context(tc.tile_pool(name="xbp", bufs=BPC * KC))
        xtp = ctx.enter_context(tc.tile_pool(name="xtp", bufs=NG * MT))
        lgp = ctx.enter_context(tc.tile_pool(name="lgp", bufs=NG))
        etp = ctx.enter_context(tc.tile_pool(name="etp", bufs=4))
        csp = ctx.enter_context(tc.tile_pool(name="csp", bufs=4))
        ctp = ctx.enter_context(tc.tile_pool(name="ctp", bufs=4 * MT))
        qsp = ctx.enter_context(tc.tile_pool(name="qsp", bufs=4))
        qtp = ctx.enter_context(tc.tile_pool(name="qtp", bufs=8))
        pbp = ctx.enter_context(tc.tile_pool(name="pbp", bufs=8))
        vtp = ctx.enter_context(tc.tile_pool(name="vtp", bufs=4))
        vmp = ctx.enter_context(tc.tile_pool(name="vmp", bufs=8))
        smp = ctx.enter_context(tc.tile_pool(name="smp", bufs=4))
        vp = ctx.enter_context(tc.tile_pool(name="vp", bufs=4))
        xsp = ctx.enter_context(tc.tile_pool(name="xsp", bufs=4))
        rzp = ctx.enter_context(tc.tile_pool(name="rzp", bufs=4))
        tinyp = ctx.enter_context(tc.tile_pool(name="tinyp", bufs=8))
        pbig = ctx.enter_context(tc.tile_pool(name="pbig", bufs=5, space="PSUM"))
        psm = ctx.enter_context(tc.tile_pool(name="psm", bufs=3, space="PSUM"))

        # --- constants ---
        w_sb = [const.tile([128, O], BF16, name=f"w{k}", tag=f"w{k}") for k in range(KC)]
        for k in range(KC):
            nc.sync.dma_start(w_sb[k][:], w_d[k])
        wt_sb = [const.tile([128, C], BF16, name=f"wt{g}", tag=f"wt{g}") for g in range(OG)]
        for g in range(OG):
            nc.sync.dma_start(wt_sb[g][:], wt_d[g])
        e16 = const.tile([D, 128], BF16, name="e16", tag="e16")
        nc.sync.dma_start(e16[:], e16_d[:])
        bm = [const.tile([128, 128], BF16, name=f"bm{g}", tag=f"bm{g}") for g in range(OG)]
        for g in range(OG):
            nc.sync.dma_start(bm[g][:], bm_d[g])
        dm4 = const.tile([128, O], F32, name="dm4", tag="dm4")
        nc.sync.dma_start(dm4[:], dm4_d[:])
        on4 = const.tile([128, GB], BF16, name="on4", tag="on4")
        nc.sync.dma_start(on4[:], on4_d[:])
        on4t = const.tile([GB, 128], BF16, name="on4t", tag="on4t")
        nc.sync.dma_start(on4t[:], on4t_d[:])
        on32 = const.tile([128, N], BF16, name="on32", tag="on32")
        nc.sync.dma_start(on32[:], on32_d[:])
        id128 = const.tile([128, 128], F32, name="id128", tag="id128")
        nc.sync.dma_start(id128[:], id128_d[:])
        cu128 = const.tile([128, 128], BF16, name="cu128", tag="cu128")
        nc.sync.dma_start(cu128[:], cu128_d[:])

        # --- x loads (group 0 first) ---
        xb = {}
        xt4 = {}
        for g in range(NG):
            for b4 in range(GB):
                b = g * GB + b4
                for k in range(KC):
                    t = xbp.tile([128, T], BF16, name="xb", tag="xb")
                    nc.sync.dma_start(t[:], xb_d[b, k])
                    xb[b, k] = t
            for m in range(MT):
                t = xtp.tile([128, GB * C], BF16, name="xt", tag="xt")
                nc.sync.dma_start(t[:], xt_d[g, m])
                xt4[g, m] = t

        logits = {g: lgp.tile([128, T], F32, name="lg", tag="lg") for g in range(NG)}

        def extract_squash(s_ps):
            """psum s_full (128(b,n), O) -> v (128, D) f32 via mask+strided reduce."""
            sm = smp.tile([128, O], F32, name="sm", tag="sm")
            nc.vector.tensor_mul(sm[:], s_ps[:], dm4[:])
            s_t = tinyp.tile([128, D], F32, name="s_t", tag="s_t")
            nc.vector.reduce_sum(
                s_t[:], sm[:].rearrange("p (n d) -> p d n", d=D), axis=AX.X
            )
            sq = tinyp.tile([128, D], F32, name="sq", tag="sq")
            nc.vector.tensor_mul(sq[:], s_t[:], s_t[:])
            s2 = tinyp.tile([128, 1], F32, name="s2", tag="s2")
            nc.vector.reduce_sum(s2[:], sq[:], axis=AX.X)
            s2e = tinyp.tile([128, 1], F32, name="s2e", tag="s2e")
            nc.vector.tensor_scalar_add(s2e[:], s2[:], EPS)
            rt = tinyp.tile([128, 1], F32, name="rt", tag="rt")
            nc.scalar.sqrt(rt[:], s2e[:])
            d1 = tinyp.tile([128, 1], F32, name="d1", tag="d1")
            nc.vector.tensor_scalar_add(d1[:], s2e[:], 1.0)
            r1 = tinyp.tile([128, 1], F32, name="r1", tag="r1")
            nc.vector.reciprocal(r1[:], d1[:])
            sc = tinyp.tile([128, 1], F32, name="sc", tag="sc")
            nc.vector.tensor_mul(sc[:], rt[:], r1[:])
            v = vp.tile([128, D], F32, name="v", tag="v")
            nc.vector.tensor_scalar_mul(v[:], s_t[:], sc[:])
            return v

        def s0_phase(g):
            """iter-0 s with uniform c via the generic Q path (cu128 lhsT)."""
            return qs_phase(g, [cu128] * MT)

        def softmax(g):
            """logits ((b,n), t) f32 -> c_stack ((b,n), t) bf16."""
            lg = logits[g]
            et = etp.tile([128, T], BF16, name="et", tag="et")
            nc.scalar.activation(et[:], lg[:], AF.Exp)
            rz = rzp.tile([GB, T], BF16, name="rz", tag="rz")
            for j in range(2):
                zs = psm.tile([GB, 512], F32, name="zs", tag="small")
                nc.tensor.matmul(
                    zs[:], on4[:], et[:, j * 512:(j + 1) * 512],
                    start=True, stop=True,
                )
                with nc.allow_low_precision(reason="bf16 softmax validated 3.5e-3"):
                    nc.vector.reciprocal(rz[:, j * 512:(j + 1) * 512], zs[:])
            cs = csp.tile([128, T], BF16, name="cs", tag="cs")
            for j in range(2):
                zb = pbig.tile([128, 512], F32, name="zb", tag="big")
                nc.tensor.matmul(
                    zb[:], on4t[:], rz[:, j * 512:(j + 1) * 512],
                    start=True, stop=True,
                )
                nc.vector.tensor_mul(
                    cs[:, j * 512:(j + 1) * 512],
                    et[:, j * 512:(j + 1) * 512], zb[:],
                )
            return cs

        def ct_phase(g, cs):
            """DMA-XBAR transpose c chunks: ((b,n), 128t) -> (128t, (b,n))."""
            cts = []
            for m in range(MT):
                ct = ctp.tile([128, 128], BF16, name="ct", tag="ct")
                nc.sync.dma_start(ct[:], cs[:, m * 128:(m + 1) * 128], transpose=True)
                cts.append(ct)
            return cts

        def qs_phase(g, cts):
            """Q = cT^T @ xT4 (diag blocks), Qt via DMA transpose, s = Qt^T W."""
            q_sb = qsp.tile([128, C], BF16, name="q_sb", tag="q_sb")
            for u in range(2):
                q_ps = pbig.tile([128, 512], F32, name="q_ps", tag="big")
                for m in range(MT):
                    nc.tensor.matmul(
                        q_ps[:], cts[m][:], xt4[g, m][:, u * 512:(u + 1) * 512],
                        start=(m == 0), stop=(m == MT - 1),
                    )
                for i in range(2):
                    b4 = u * 2 + i
                    nc.vector.tensor_copy(
                        q_sb[32 * b4:32 * (b4 + 1), :],
                        q_ps[32 * b4:32 * (b4 + 1), i * C:(i + 1) * C],
                    )
            qts = []
            for h in range(KC):
                qt = qtp.tile([128, 128], BF16, name="qt", tag="qt")
                nc.sync.dma_start(qt[:], q_sb[:, h * 128:(h + 1) * 128], transpose=True)
                qts.append(qt)
            s_ps = pbig.tile([128, O], F32, name="s_ps", tag="big")
            for h in range(KC):
                nc.tensor.matmul(
                    s_ps[:], qts[h][:], w_sb[h][:],
                    start=(h == 0), stop=(h == KC - 1),
                )
            return s_ps

        def update(g, v, first):
            """logits ((b,n), t) += x^T (W . vmask) for the 4 stacked batches."""
            vt_ps = psm.tile([D, 128], F32, name="vt_ps", tag="small")
            nc.tensor.transpose(vt_ps[:], v[:], id128[:])
            vt_bf = vtp.tile([D, 128], BF16, name="vt_bf", tag="vt_bf")
            nc.vector.tensor_copy(vt_bf[:], vt_ps[:])
            vbc_ps = psm.tile([128, 128], F32, name="vbc", tag="small")
            nc.tensor.matmul(vbc_ps[:], e16[:], vt_bf[:], start=True, stop=True)
            vms = []
            for g4 in range(OG):
                vm = vmp.tile([128, 128], BF16, name="vm", tag="vm")
                nc.vector.tensor_mul(vm[:], vbc_ps[:], bm[g4][:])
                vms.append(vm)
            p_sb = []
            for h in range(KC):
                p_ps = psm.tile([128, 128], F32, name="p_ps", tag="small")
                for g4 in range(OG):
                    nc.tensor.matmul(
                        p_ps[:], wt_sb[g4][:, h * 128:(h + 1) * 128], vms[g4][:],
                        start=(g4 == 0), stop=(g4 == OG - 1),
                    )
                pb = pbp.tile([128, 128], BF16, name="pb", tag="pb")
                nc.vector.tensor_copy(pb[:], p_ps[:])
                p_sb.append(pb)
            lg = logits[g]
            for j in range(2):
                a_ps = pbig.tile([128, 512], F32, name="a_ps", tag="big")
                for b4 in range(GB):
                    for k in range(KC):
                        nc.tensor.matmul(
                            a_ps[32 * b4:32 * (b4 + 1), :],
                            p_sb[k][:, 32 * b4:32 * (b4 + 1)],
                            xb[g * GB + b4, k][:, j * 512:(j + 1) * 512],
                            start=(k == 0), stop=(k == KC - 1),
                            tile_position=(0, 32 * b4),
                        )
                if first:
                    nc.scalar.copy(lg[:, j * 512:(j + 1) * 512], a_ps[:])
                else:
                    nc.vector.tensor_add(
                        lg[:, j * 512:(j + 1) * 512],
                        lg[:, j * 512:(j + 1) * 512], a_ps[:],
                    )

        # --- iteration 0 (uniform c) ---
        vs = {}
        sps = {g: s0_phase(g) for g in range(NG)}
        for g in range(NG):
            vs[g] = extract_squash(sps[g])
        for g in range(NG):
            update(g, vs[g], first=True)

        # --- iterations 1, 2 ---
        for it in (1, 2):
            css = {g: softmax(g) for g in range(NG)}
            ctss = {g: ct_phase(g, css[g]) for g in range(NG)}
            sps = {g: qs_phase(g, ctss[g]) for g in range(NG)}
            for g in range(NG):
                vs[g] = extract_squash(sps[g])
            if it == 1:
                for g in range(NG):
                    update(g, vs[g], first=False)
            else:
                for g in range(NG):
                    for b4 in range(GB):
                        nc.sync.dma_start(
                            out_d[g * GB + b4],
                            vs[g][32 * b4:32 * (b4 + 1), :],
                        )


_NC_CACHE = {}


def _get_nc():
    if "nc" not in _NC_CACHE:
        _NC_CACHE["nc"] = _build_bass()
    return _NC_CACHE["nc"]


def _make_in_maps(x, W):
    BFnp = ml_dtypes.bfloat16
    x = np.asarray(x, np.float32)
    W = np.asarray(W, np.float32)
    w_bf = np.ascontiguousarray(W.reshape(KC, 128, O)).astype(BFnp)
    wt = np.ascontiguousarray(W.reshape(C, OG, 128).transpose(1, 2, 0)).astype(BFnp)
    e16 = (np.arange(128)[None, :] % D == np.arange(D)[:, None]).astype(BFnp)
    oo = np.arange(128)
    bn = np.arange(128)
    bm = np.stack(
        [((g * 8 + oo[:, None] // D) == (bn[None, :] % N)) for g in range(OG)]
    ).astype(BFnp)
    dm4 = ((np.arange(O)[None, :] // D) == (bn[:, None] % N)).astype(np.float32)
    on4 = (bn[:, None] // N == np.arange(GB)[None, :]).astype(BFnp)
    on4t = np.ascontiguousarray(on4.T).astype(BFnp)
    on32 = np.full((128, N), 1.0 / N, BFnp)
    id128 = np.eye(128, dtype=np.float32)
    cu128 = np.full((128, 128), 1.0 / N, BFnp)

    in_maps = []
    for core in range(NCORES):
        xs = x[core * BPC:(core + 1) * BPC]              # (8, C, T)
        xbt = np.ascontiguousarray(xs.reshape(BPC, KC, 128, T)).astype(BFnp)
        xt4 = np.zeros((NG, MT, 128, GB * C), BFnp)
        for g in range(NG):
            for b4 in range(GB):
                xtb = xs[g * GB + b4].T                  # (T, C) f32
                xt4[g, :, :, b4 * C:(b4 + 1) * C] = (
                    xtb.reshape(MT, 128, C).astype(BFnp)
                )
        in_maps.append(
            {
                "xb": xbt, "xt4": xt4, "wsb": w_bf, "wt": wt, "e16": e16,
                "bm": bm, "dm4": dm4, "on4": on4, "on4t": on4t,
                "on32": on32, "id128": id128, "cu128": cu128,
            }
        )
    return in_maps


def run(x, W, trace=False):
    in_maps = _make_in_maps(x, W)
    nc = _get_nc()
    res = run_bass_kernel_spmd(nc, in_maps, core_ids=list(range(NCORES)), trace=trace)
    out = np.concatenate([r["out"] for r in res.results], axis=0)
    return out, res


def kernel(x, W, out_num_capsule=N, out_dim_capsule=D, routings=3, **_):
    out, _res = run(x, W, trace=False)
    return out
